# revision 5
# baseline (speedup 1.0000x reference)
"""Trainium2 Bass kernel for CustomSAGEConv (GNN mean-aggregation message passing).

  out = normalize( mean_agg(x[row] -> col) @ W_agg.T + x @ W_lin.T )

Strategy (8 NeuronCores, SPMD single program):
  - Host: partition the 100K nodes into 784 blocks of 128 (8 cores x 98 blocks),
    balancing block in-degree via degree-sorted snake round-robin (+repair) so
    every block has <= 2048 incoming edges -> M=16 chunks of 128 edges/block.
    Edges are routed to the core/block owning their destination (col); within a
    block they are padded to M*128 slots (dummy slots get loc=255 -> no-op).
    Host also precomputes 1/max(indegree,1) (metadata, like the partitioning).
  - Device, per block b:
      1. M indirect-DMA gathers of 128 source rows each from replicated x,
         all into one [128, M, 128] tile (single pool-wrap wait per block;
         the SWDGE descriptor pump on the Q7 is the kernel's hard bottleneck
         at ~1.17us/128 rows, so GpSimd-engine overhead is minimized).
      2. build one-hot S[e, m, c] = (loc[e, m] == c) with one broadcast
         is_equal (DVE).
      3. M matmuls accumulate PSUM[c, :] += S_m.T @ msgs_m (node-major sums).
      4. agg = summed * invdeg  (ACT copy-with-scale)
      5. PE-transpose agg -> agg_T; out = agg_T.T @ W_agg.T + x_T.T @ W_lin.T
      6. row L2-normalize (ACT square+accum / sqrt, DVE max/recip, ACT scale).
  - Host: inverse-permute rows back to original node order.

Perf notes (HW-measured): the gather MUST go through gpsimd indirect DMA with
a [128, 1] offset column per call -- multi-column offset APs lower incorrectly
through walrus (probed: descriptors get one offset per partition + contiguous
walk), and the Ant dma_gather custom instruction needs HIPI ucode that this
runtime (bedrock) does not ship. 1568 calls x ~1.17us Q7 descriptor
generation ~= 1.84ms/core is therefore the floor of this design; everything
else overlaps under it.
"""

import sys

sys.path.insert(0, "/opt/trn_rl_repo")

import numpy as np

P = 128
GB = 7  # blocks per xt batch load


# ---------------------------------------------------------------- host prep

def _host_prep(x, W_lin, W_agg, edge_index, ncores, bpc, dt_np):
    """Build per-core device inputs. Returns (in_maps, node_of_slot, M)."""
    N, D = x.shape
    assert D == P
    NBLK = ncores * bpc
    NPAD = NBLK * P
    assert N <= NPAD

    row = np.ascontiguousarray(edge_index[0]).astype(np.int32)
    col = np.ascontiguousarray(edge_index[1]).astype(np.int32)

    # --- balanced node->block assignment (degree-sorted snake round robin)
    deg = np.bincount(col, minlength=NPAD).astype(np.int64)
    order = np.argsort(-deg, kind="stable")
    seq = np.arange(NPAD, dtype=np.int64)
    cyc, pos = seq // NBLK, seq % NBLK
    snake = np.where(cyc % 2 == 0, pos, NBLK - 1 - pos).astype(np.int32)
    blk_of = np.empty(NPAD, np.int32)
    blk_of[order] = snake
    sums = np.bincount(blk_of[col], minlength=NBLK).astype(np.int64)
    CAP = 2048
    for _ in range(1000):
        if sums.max() <= CAP:
            break
        b_hi = int(np.argmax(sums))
        b_lo = int(np.argmin(sums))
        need = sums[b_hi] - CAP
        nodes_hi = np.where(blk_of == b_hi)[0]
        nodes_lo = np.where(blk_of == b_lo)[0]
        n2 = nodes_lo[np.argmin(deg[nodes_lo])]
        cand = nodes_hi[deg[nodes_hi] >= deg[n2] + need]
        if len(cand) == 0:
            cand = nodes_hi[np.argmax(deg[nodes_hi])][None]
        n1 = cand[np.argmin(deg[cand])]
        blk_of[n1], blk_of[n2] = b_lo, b_hi
        d = deg[n1] - deg[n2]
        sums[b_hi] -= d
        sums[b_lo] += d
    M = max(1, int(np.ceil(sums.max() / P)))

    # loc within block + slot->node map
    o2 = np.argsort(blk_of, kind="stable")  # nodes grouped by block (128 each)
    loc_of = np.empty(NPAD, np.int32)
    loc_of[o2] = (np.arange(NPAD, dtype=np.int64) % P).astype(np.int32)
    node_of_slot = o2  # global slot (blk*128+loc) -> node id

    # --- edge slot arrays
    eb = blk_of[col]
    el = loc_of[col]
    eo = np.lexsort((row, eb))  # group by block, sort by source row (locality)
    eb_s, row_s, el_s = eb[eo], row[eo], el[eo]
    cnt = np.bincount(eb_s, minlength=NBLK)
    starts = np.concatenate([[0], np.cumsum(cnt)[:-1]])
    SLOTS = M * P
    rows_slots = np.zeros((NBLK, SLOTS), np.int32)
    locs_slots = np.full((NBLK, SLOTS), 255.0, np.float32)
    within = np.arange(len(eo), dtype=np.int64) - np.repeat(starts, cnt)
    flat = eb_s.astype(np.int64) * SLOTS + within
    rows_slots.reshape(-1)[flat] = row_s
    locs_slots.reshape(-1)[flat] = el_s

    # device layout [core, partition(e), block*M + m]
    rows_T = np.ascontiguousarray(
        rows_slots.reshape(ncores, bpc, M, P).transpose(0, 3, 1, 2)
    ).reshape(ncores, P, bpc * M)
    locs_T = np.ascontiguousarray(
        locs_slots.reshape(ncores, bpc, M, P).transpose(0, 3, 1, 2)
    ).reshape(ncores, P, bpc * M).astype(dt_np)

    # gather table
    xg = np.ascontiguousarray(x.astype(dt_np))

    # inverse in-degree per (core, loc, block)  [deg of node at slot]
    invdeg = (1.0 / np.maximum(deg, 1.0)).astype(np.float32)
    invdeg_slot = invdeg[node_of_slot]  # [NPAD] slot order
    invdeg_T = np.ascontiguousarray(
        invdeg_slot.reshape(ncores, bpc, P).transpose(0, 2, 1))  # [k, loc, blk]

    # per-core transposed x in slot order
    x_pad = np.zeros((NPAD, P), np.float32)
    x_pad[:N] = x
    xt_all = x_pad[node_of_slot].astype(dt_np)  # [NPAD, 128] slot order
    xt_cores = np.ascontiguousarray(
        xt_all.reshape(ncores, bpc * P, P).transpose(0, 2, 1)
    )  # [k, 128, bpc*128]

    waggT = np.ascontiguousarray(W_agg.T).astype(dt_np)
    wlinT = np.ascontiguousarray(W_lin.T).astype(dt_np)
    iota = np.tile(np.arange(P, dtype=np.float64), (P, 1)).astype(dt_np)
    ident = np.eye(P, dtype=np.float64).astype(dt_np)

    in_maps = []
    for k in range(ncores):
        in_maps.append({
            "xg": xg,
            "xt": xt_cores[k],
            "wagg": waggT,
            "wlin": wlinT,
            "rows": rows_T[k],
            "locs": locs_T[k],
            "invdeg": invdeg_T[k],
            "iota": iota,
            "ident": ident,
        })
    return in_maps, node_of_slot, M


# ---------------------------------------------------------------- device program

def _build_nc(bpc, M, dt_np, n_table_rows, debug=False):
    import concourse.bass as bass
    import concourse.bacc as bacc
    import concourse.mybir as mybir
    import concourse.tile as tile

    dt = mybir.dt.from_np(np.dtype(dt_np))
    f32 = mybir.dt.float32
    NB = bpc
    NCN = NB * P

    nc = bacc.Bacc("TRN2", target_bir_lowering=False, debug=debug)

    xg_d = nc.dram_tensor("xg", [n_table_rows, P], dt, kind="ExternalInput")
    xt_d = nc.dram_tensor("xt", [P, NCN], dt, kind="ExternalInput")
    wagg_d = nc.dram_tensor("wagg", [P, P], dt, kind="ExternalInput")
    wlin_d = nc.dram_tensor("wlin", [P, P], dt, kind="ExternalInput")
    rows_d = nc.dram_tensor("rows", [P, NB * M], mybir.dt.int32, kind="ExternalInput")
    locs_d = nc.dram_tensor("locs", [P, NB * M], dt, kind="ExternalInput")
    invdeg_d = nc.dram_tensor("invdeg", [P, NB], f32, kind="ExternalInput")
    iota_d = nc.dram_tensor("iota", [P, P], dt, kind="ExternalInput")
    ident_d = nc.dram_tensor("ident", [P, P], dt, kind="ExternalInput")
    out_d = nc.dram_tensor("out", [NCN, P], f32, kind="ExternalOutput")

    AF = mybir.ActivationFunctionType
    OP = mybir.AluOpType

    with tile.TileContext(nc) as tc:
        with tc.tile_pool(name="const", bufs=1) as cp, \
             tc.tile_pool(name="msg", bufs=6) as mp, \
             tc.tile_pool(name="spool", bufs=4) as spool, \
             tc.tile_pool(name="xtp", bufs=2) as xp, \
             tc.tile_pool(name="blk", bufs=3) as bp, \
             tc.tile_pool(name="psum", bufs=2, space="PSUM") as pp:

            # rows first: gathers depend only on this
            rows_t = cp.tile([P, NB * M], mybir.dt.int32)
            nc.sync.dma_start(out=rows_t[:], in_=rows_d[:])
            locs_t = cp.tile([P, NB * M], dt)
            nc.sync.dma_start(out=locs_t[:], in_=locs_d[:])
            invdeg_t = cp.tile([P, NB], f32)
            nc.sync.dma_start(out=invdeg_t[:], in_=invdeg_d[:])
            iota_t = cp.tile([P, P], dt)
            nc.sync.dma_start(out=iota_t[:], in_=iota_d[:])
            ident_t = cp.tile([P, P], dt)
            nc.sync.dma_start(out=ident_t[:], in_=ident_d[:])
            wagg_t = cp.tile([P, P], dt)
            nc.sync.dma_start(out=wagg_t[:], in_=wagg_d[:])
            wlin_t = cp.tile([P, P], dt)
            nc.sync.dma_start(out=wlin_t[:], in_=wlin_d[:])

            for b in range(NB):
                # 1. M per-chunk gathers into one block tile (slices are
                # disjoint; same-engine program order needs no semaphores)
                msg_t = mp.tile([P, M, P], dt, tag="msg")
                for m in range(M):
                    nc.gpsimd.indirect_dma_start(
                        out=msg_t[:, m, :], out_offset=None, in_=xg_d[:],
                        in_offset=bass.IndirectOffsetOnAxis(
                            ap=rows_t[:, b * M + m:b * M + m + 1], axis=0))

                if b % GB == 0:
                    xt_t = xp.tile([P, GB * P], dt, tag="xt")
                    nc.sync.dma_start(
                        out=xt_t[:], in_=xt_d[:, b * P:(b + GB) * P])

                # 2. one-hot S[e, m, c] = (loc[e, m] == c)
                S_t = spool.tile([P, M, P], dt, tag="S")
                nc.vector.tensor_tensor(
                    out=S_t[:],
                    in0=locs_t[:, b * M:(b + 1) * M].to_broadcast([P, M, P]),
                    in1=iota_t[:, None, :].to_broadcast([P, M, P]),
                    op=OP.is_equal)

                # 3. scatter-accumulate: acc[c, :] += S_m.T @ msgs_m
                acc_p = pp.tile([P, P], f32, tag="acc")
                for m in range(M):
                    nc.tensor.matmul(
                        out=acc_p[:], lhsT=S_t[:, m, :], rhs=msg_t[:, m, :],
                        start=(m == 0), stop=(m == M - 1))

                # 4. agg = summed * invdeg  (ACT copy-with-scale)
                agg_t = bp.tile([P, P], dt, tag="agg")
                nc.scalar.mul(agg_t[:], acc_p[:], invdeg_t[:, b:b + 1])

                # 5. transpose agg; project: out = agg @ W_agg.T + x @ W_lin.T
                aggT_p = pp.tile([P, P], dt, tag="aggTp")
                nc.tensor.transpose(out=aggT_p[:], in_=agg_t[:],
                                    identity=ident_t[:])
                aggT_t = bp.tile([P, P], dt, tag="aggT")
                nc.scalar.copy(aggT_t[:], aggT_p[:])
                out_p = pp.tile([P, P], f32, tag="out")
                nc.tensor.matmul(out=out_p[:], lhsT=aggT_t[:], rhs=wagg_t[:],
                                 start=True, stop=False)
                nc.tensor.matmul(out=out_p[:],
                                 lhsT=xt_t[:, (b % GB) * P:(b % GB + 1) * P],
                                 rhs=wlin_t[:], start=False, stop=True)

                # 6. L2 normalize rows
                sq_t = bp.tile([P, P], f32, tag="sq")
                ss_t = bp.tile([P, 1], f32, tag="ss")
                nc.scalar.activation(out=sq_t[:], in_=out_p[:], func=AF.Square,
                                     accum_out=ss_t[:])
                nrm_t = bp.tile([P, 1], f32, tag="nrm")
                nc.scalar.sqrt(out=nrm_t[:], in_=ss_t[:])
                nrmc_t = bp.tile([P, 1], f32, tag="nrmc")
                nc.vector.tensor_scalar_max(nrmc_t[:], nrm_t[:], 1e-12)
                inv_t = bp.tile([P, 1], f32, tag="inv")
                nc.vector.reciprocal(out=inv_t[:], in_=nrmc_t[:])
                outs_t = bp.tile([P, P], f32, tag="outs")
                nc.scalar.mul(outs_t[:], out_p[:], inv_t[:, :1])
                nc.sync.dma_start(out=out_d[b * P:(b + 1) * P, :],
                                  in_=outs_t[:])

    return nc


# ---------------------------------------------------------------- entry point

def _run(x, W_lin, W_agg, edge_index, ncores, bpc, dt_np, trace=False):
    from concourse import bass_utils

    in_maps, node_of_slot, M = _host_prep(
        x, W_lin, W_agg, edge_index, ncores, bpc, dt_np)
    nc = _build_nc(bpc, M, dt_np, in_maps[0]["xg"].shape[0])
    nc.compile()
    res = bass_utils.run_bass_kernel_spmd(
        nc, in_maps, core_ids=list(range(ncores)), trace=trace)
    outs = np.concatenate([r["out"] for r in res.results], axis=0)
    N = x.shape[0]
    out_pad = np.empty((len(node_of_slot), P), np.float32)
    out_pad[node_of_slot] = outs
    return out_pad[:N], res


def kernel(x, W_lin, W_agg, edge_index):
    import os
    x = np.ascontiguousarray(x, dtype=np.float32)
    W_lin = np.ascontiguousarray(W_lin, dtype=np.float32)
    W_agg = np.ascontiguousarray(W_agg, dtype=np.float32)
    dt_np = np.float16
    trace = os.environ.get("KERNEL_TRACE", "0") == "1"
    if trace:
        try:
            sys.path.insert(0, os.path.dirname(os.path.abspath(__file__)))
            import ntff_shim  # noqa: F401
        except Exception:
            pass
    out, res = _run(x, W_lin, W_agg, edge_index, ncores=8, bpc=98,
                    dt_np=dt_np, trace=trace)
    if res.exec_time_ns is not None:
        print(f"HW exec time: {res.exec_time_ns} ns")
    return out


# revision 6
# speedup vs baseline: 1.0079x; 1.0079x over previous
"""Trainium2 Bass kernel for CustomSAGEConv (GNN mean-aggregation message passing).

  out = normalize( mean_agg(x[row] -> col) @ W_agg.T + x @ W_lin.T )

Strategy (8 NeuronCores, SPMD single program):
  - Host: partition the 100K nodes into 784 blocks of 128 (8 cores x 98 blocks),
    balancing block in-degree via degree-sorted snake round-robin (+repair) so
    every block has <= 2048 incoming edges -> M=16 chunks of 128 edges/block.
    Edges are routed to the core/block owning their destination (col); within a
    block they are padded to M*128 slots (dummy slots get loc=255 -> no-op).
    Host also precomputes 1/max(indegree,1) (metadata, like the partitioning).
  - Device, per block b:
      1. M indirect-DMA gathers of 128 source rows each from replicated x,
         all into one [128, M, 128] tile (single pool-wrap wait per block;
         the SWDGE descriptor pump on the Q7 is the kernel's hard bottleneck
         at ~1.17us/128 rows, so GpSimd-engine overhead is minimized).
      2. build one-hot S[e, m, c] = (loc[e, m] == c) with one broadcast
         is_equal (DVE).
      3. M matmuls accumulate PSUM[c, :] += S_m.T @ msgs_m (node-major sums).
      4. agg = summed * invdeg  (ACT copy-with-scale)
      5. PE-transpose agg -> agg_T; out = agg_T.T @ W_agg.T + x_T.T @ W_lin.T
      6. row L2-normalize (ACT square+accum / sqrt, DVE max/recip, ACT scale).
  - Host: inverse-permute rows back to original node order.

Perf notes (HW-measured): the gather MUST go through gpsimd indirect DMA with
a [128, 1] offset column per call -- multi-column offset APs lower incorrectly
through walrus (probed: descriptors get one offset per partition + contiguous
walk), and the Ant dma_gather custom instruction needs HIPI ucode that this
runtime (bedrock) does not ship. 1568 calls x ~1.17us Q7 descriptor
generation ~= 1.84ms/core is therefore the floor of this design; everything
else overlaps under it.
"""

import sys

sys.path.insert(0, "/opt/trn_rl_repo")

import numpy as np

P = 128
GB = 7  # blocks per xt batch load


# ---------------------------------------------------------------- host prep

def _host_prep(x, W_lin, W_agg, edge_index, ncores, bpc, dt_np):
    """Build per-core device inputs. Returns (in_maps, node_of_slot, M)."""
    N, D = x.shape
    assert D == P
    NBLK = ncores * bpc
    NPAD = NBLK * P
    assert N <= NPAD

    row = np.ascontiguousarray(edge_index[0]).astype(np.int32)
    col = np.ascontiguousarray(edge_index[1]).astype(np.int32)

    # --- balanced node->block assignment (degree-sorted snake round robin)
    deg = np.bincount(col, minlength=NPAD).astype(np.int64)
    order = np.argsort(-deg, kind="stable")
    seq = np.arange(NPAD, dtype=np.int64)
    cyc, pos = seq // NBLK, seq % NBLK
    snake = np.where(cyc % 2 == 0, pos, NBLK - 1 - pos).astype(np.int32)
    blk_of = np.empty(NPAD, np.int32)
    blk_of[order] = snake
    sums = np.bincount(blk_of[col], minlength=NBLK).astype(np.int64)
    CAP = 2048
    for _ in range(1000):
        if sums.max() <= CAP:
            break
        b_hi = int(np.argmax(sums))
        b_lo = int(np.argmin(sums))
        need = sums[b_hi] - CAP
        nodes_hi = np.where(blk_of == b_hi)[0]
        nodes_lo = np.where(blk_of == b_lo)[0]
        n2 = nodes_lo[np.argmin(deg[nodes_lo])]
        cand = nodes_hi[deg[nodes_hi] >= deg[n2] + need]
        if len(cand) == 0:
            cand = nodes_hi[np.argmax(deg[nodes_hi])][None]
        n1 = cand[np.argmin(deg[cand])]
        blk_of[n1], blk_of[n2] = b_lo, b_hi
        d = deg[n1] - deg[n2]
        sums[b_hi] -= d
        sums[b_lo] += d
    M = max(1, int(np.ceil(sums.max() / P)))

    # loc within block + slot->node map
    o2 = np.argsort(blk_of, kind="stable")  # nodes grouped by block (128 each)
    loc_of = np.empty(NPAD, np.int32)
    loc_of[o2] = (np.arange(NPAD, dtype=np.int64) % P).astype(np.int32)
    node_of_slot = o2  # global slot (blk*128+loc) -> node id

    # --- edge slot arrays
    eb = blk_of[col]
    el = loc_of[col]
    eo = np.lexsort((row, eb))  # group by block, sort by source row (locality)
    eb_s, row_s, el_s = eb[eo], row[eo], el[eo]
    cnt = np.bincount(eb_s, minlength=NBLK)
    starts = np.concatenate([[0], np.cumsum(cnt)[:-1]])
    SLOTS = M * P
    rows_slots = np.zeros((NBLK, SLOTS), np.int32)
    locs_slots = np.full((NBLK, SLOTS), 255.0, np.float32)
    within = np.arange(len(eo), dtype=np.int64) - np.repeat(starts, cnt)
    flat = eb_s.astype(np.int64) * SLOTS + within
    rows_slots.reshape(-1)[flat] = row_s
    locs_slots.reshape(-1)[flat] = el_s

    # device layout [core, partition(e), block*M + m]
    rows_T = np.ascontiguousarray(
        rows_slots.reshape(ncores, bpc, M, P).transpose(0, 3, 1, 2)
    ).reshape(ncores, P, bpc * M)
    locs_T = np.ascontiguousarray(
        locs_slots.reshape(ncores, bpc, M, P).transpose(0, 3, 1, 2)
    ).reshape(ncores, P, bpc * M).astype(dt_np)

    # gather table
    xg = np.ascontiguousarray(x.astype(dt_np))

    # inverse in-degree per (core, loc, block)  [deg of node at slot]
    invdeg = (1.0 / np.maximum(deg, 1.0)).astype(np.float32)
    invdeg_slot = invdeg[node_of_slot]  # [NPAD] slot order
    invdeg_T = np.ascontiguousarray(
        invdeg_slot.reshape(ncores, bpc, P).transpose(0, 2, 1))  # [k, loc, blk]

    # per-core transposed x in slot order
    x_pad = np.zeros((NPAD, P), np.float32)
    x_pad[:N] = x
    xt_all = x_pad[node_of_slot].astype(dt_np)  # [NPAD, 128] slot order
    xt_cores = np.ascontiguousarray(
        xt_all.reshape(ncores, bpc * P, P).transpose(0, 2, 1)
    )  # [k, 128, bpc*128]

    waggT = np.ascontiguousarray(W_agg.T).astype(dt_np)
    wlinT = np.ascontiguousarray(W_lin.T).astype(dt_np)
    iota = np.tile(np.arange(P, dtype=np.float64), (P, 1)).astype(dt_np)
    ident = np.eye(P, dtype=np.float64).astype(dt_np)

    in_maps = []
    for k in range(ncores):
        in_maps.append({
            "xg": xg,
            "xt": xt_cores[k],
            "wagg": waggT,
            "wlin": wlinT,
            "rows": rows_T[k],
            "locs": locs_T[k],
            "invdeg": invdeg_T[k],
            "iota": iota,
            "ident": ident,
        })
    return in_maps, node_of_slot, M


# ---------------------------------------------------------------- device program

def _build_nc(bpc, M, dt_np, n_table_rows, debug=False):
    import concourse.bass as bass
    import concourse.bacc as bacc
    import concourse.mybir as mybir
    import concourse.tile as tile

    dt = mybir.dt.from_np(np.dtype(dt_np))
    f32 = mybir.dt.float32
    NB = bpc
    NCN = NB * P

    nc = bacc.Bacc("TRN2", target_bir_lowering=False, debug=debug)

    xg_d = nc.dram_tensor("xg", [n_table_rows, P], dt, kind="ExternalInput")
    xt_d = nc.dram_tensor("xt", [P, NCN], dt, kind="ExternalInput")
    wagg_d = nc.dram_tensor("wagg", [P, P], dt, kind="ExternalInput")
    wlin_d = nc.dram_tensor("wlin", [P, P], dt, kind="ExternalInput")
    rows_d = nc.dram_tensor("rows", [P, NB * M], mybir.dt.int32, kind="ExternalInput")
    locs_d = nc.dram_tensor("locs", [P, NB * M], dt, kind="ExternalInput")
    invdeg_d = nc.dram_tensor("invdeg", [P, NB], f32, kind="ExternalInput")
    iota_d = nc.dram_tensor("iota", [P, P], dt, kind="ExternalInput")
    ident_d = nc.dram_tensor("ident", [P, P], dt, kind="ExternalInput")
    out_d = nc.dram_tensor("out", [NCN, P], f32, kind="ExternalOutput")

    AF = mybir.ActivationFunctionType
    OP = mybir.AluOpType

    with tile.TileContext(nc) as tc:
        with tc.tile_pool(name="const", bufs=1) as cp, \
             tc.tile_pool(name="msg", bufs=4) as mp, \
             tc.tile_pool(name="spool", bufs=4) as spool, \
             tc.tile_pool(name="xtp", bufs=2) as xp, \
             tc.tile_pool(name="blk", bufs=3) as bp, \
             tc.tile_pool(name="psum", bufs=2, space="PSUM") as pp:

            # rows first: gathers depend only on this
            rows_t = cp.tile([P, NB * M], mybir.dt.int32)
            nc.sync.dma_start(out=rows_t[:], in_=rows_d[:])
            locs_t = cp.tile([P, NB * M], dt)
            nc.sync.dma_start(out=locs_t[:], in_=locs_d[:])
            invdeg_t = cp.tile([P, NB], f32)
            nc.sync.dma_start(out=invdeg_t[:], in_=invdeg_d[:])
            iota_t = cp.tile([P, P], dt)
            nc.sync.dma_start(out=iota_t[:], in_=iota_d[:])
            ident_t = cp.tile([P, P], dt)
            nc.sync.dma_start(out=ident_t[:], in_=ident_d[:])
            wagg_t = cp.tile([P, P], dt)
            nc.sync.dma_start(out=wagg_t[:], in_=wagg_d[:])
            wlin_t = cp.tile([P, P], dt)
            nc.sync.dma_start(out=wlin_t[:], in_=wlin_d[:])

            for b in range(NB):
                # 1. M per-chunk gathers into one block tile (slices are
                # disjoint; same-engine program order needs no semaphores)
                msg_t = mp.tile([P, M, P], dt, tag="msg")
                for m in range(M):
                    nc.gpsimd.indirect_dma_start(
                        out=msg_t[:, m, :], out_offset=None, in_=xg_d[:],
                        in_offset=bass.IndirectOffsetOnAxis(
                            ap=rows_t[:, b * M + m:b * M + m + 1], axis=0))

                if b % GB == 0:
                    xt_t = xp.tile([P, GB * P], dt, tag="xt")
                    nc.sync.dma_start(
                        out=xt_t[:], in_=xt_d[:, b * P:(b + GB) * P])

                # 2. one-hot S[e, m, c] = (loc[e, m] == c)
                S_t = spool.tile([P, M, P], dt, tag="S")
                nc.vector.tensor_tensor(
                    out=S_t[:],
                    in0=locs_t[:, b * M:(b + 1) * M].to_broadcast([P, M, P]),
                    in1=iota_t[:, None, :].to_broadcast([P, M, P]),
                    op=OP.is_equal)

                # 3. scatter-accumulate: acc[c, :] += S_m.T @ msgs_m
                acc_p = pp.tile([P, P], f32, tag="acc")
                for m in range(M):
                    nc.tensor.matmul(
                        out=acc_p[:], lhsT=S_t[:, m, :], rhs=msg_t[:, m, :],
                        start=(m == 0), stop=(m == M - 1))

                # 4. agg = summed * invdeg  (ACT copy-with-scale)
                agg_t = bp.tile([P, P], dt, tag="agg")
                nc.scalar.mul(agg_t[:], acc_p[:], invdeg_t[:, b:b + 1])

                # 5. transpose agg; project: out = agg @ W_agg.T + x @ W_lin.T
                aggT_p = pp.tile([P, P], dt, tag="aggTp")
                nc.tensor.transpose(out=aggT_p[:], in_=agg_t[:],
                                    identity=ident_t[:])
                aggT_t = bp.tile([P, P], dt, tag="aggT")
                nc.scalar.copy(aggT_t[:], aggT_p[:])
                out_p = pp.tile([P, P], f32, tag="out")
                nc.tensor.matmul(out=out_p[:], lhsT=aggT_t[:], rhs=wagg_t[:],
                                 start=True, stop=False)
                nc.tensor.matmul(out=out_p[:],
                                 lhsT=xt_t[:, (b % GB) * P:(b % GB + 1) * P],
                                 rhs=wlin_t[:], start=False, stop=True)

                # 6. L2 normalize rows
                sq_t = bp.tile([P, P], f32, tag="sq")
                ss_t = bp.tile([P, 1], f32, tag="ss")
                nc.scalar.activation(out=sq_t[:], in_=out_p[:], func=AF.Square,
                                     accum_out=ss_t[:])
                nrm_t = bp.tile([P, 1], f32, tag="nrm")
                nc.scalar.sqrt(out=nrm_t[:], in_=ss_t[:])
                nrmc_t = bp.tile([P, 1], f32, tag="nrmc")
                nc.vector.tensor_scalar_max(nrmc_t[:], nrm_t[:], 1e-12)
                inv_t = bp.tile([P, 1], f32, tag="inv")
                nc.vector.reciprocal(out=inv_t[:], in_=nrmc_t[:])
                outs_t = bp.tile([P, P], f32, tag="outs")
                nc.scalar.mul(outs_t[:], out_p[:], inv_t[:, :1])
                nc.sync.dma_start(out=out_d[b * P:(b + 1) * P, :],
                                  in_=outs_t[:])

    return nc


# ---------------------------------------------------------------- entry point

def _run(x, W_lin, W_agg, edge_index, ncores, bpc, dt_np, trace=False):
    from concourse import bass_utils

    in_maps, node_of_slot, M = _host_prep(
        x, W_lin, W_agg, edge_index, ncores, bpc, dt_np)
    nc = _build_nc(bpc, M, dt_np, in_maps[0]["xg"].shape[0])
    nc.compile()
    res = bass_utils.run_bass_kernel_spmd(
        nc, in_maps, core_ids=list(range(ncores)), trace=trace)
    outs = np.concatenate([r["out"] for r in res.results], axis=0)
    N = x.shape[0]
    out_pad = np.empty((len(node_of_slot), P), np.float32)
    out_pad[node_of_slot] = outs
    return out_pad[:N], res


def kernel(x, W_lin, W_agg, edge_index):
    import os
    x = np.ascontiguousarray(x, dtype=np.float32)
    W_lin = np.ascontiguousarray(W_lin, dtype=np.float32)
    W_agg = np.ascontiguousarray(W_agg, dtype=np.float32)
    dt_np = np.float16
    trace = os.environ.get("KERNEL_TRACE", "0") == "1"
    if trace:
        try:
            sys.path.insert(0, os.path.dirname(os.path.abspath(__file__)))
            import ntff_shim  # noqa: F401
        except Exception:
            pass
    out, res = _run(x, W_lin, W_agg, edge_index, ncores=8, bpc=98,
                    dt_np=dt_np, trace=trace)
    if res.exec_time_ns is not None:
        print(f"HW exec time: {res.exec_time_ns} ns")
    return out
